# revision 10
# baseline (speedup 1.0000x reference)
"""Trainium2 Bass kernel for nn_Concat_Linear (feat [65536,2,768] -> out [65536,9]).

Data-parallel across 8 NeuronCores (8192 rows each). Per core, fp32 throughout:
  - natural-layout DMA loads (full HBM rate), PE-transpose 128x128 chunks to get
    the feature dim onto partitions, then 12 accumulating matmuls against a
    host-prescrambled weight tile produce Y = [this | last] in [41, 512] PSUM
    (this at partitions 0:9, last at 32:41 so both are 32-aligned matmul inputs).
  - the trilinear form, LayerNorm and final linear run in "feature-on-partition"
    orientation via small PE matmuls + DVE/ACT elementwise ops; ln_w/ln_b are
    folded host-side into the final weights/bias; rstd = exp(-0.5*ln(var+eps)).
  - outputs are PE-transposed back to row-major and stored once per 512 rows.
"""

import sys
import types

import numpy as np

B_FULL = 65536
N_CORES = 8
B_CORE = B_FULL // N_CORES
D = 1536  # 2 * 768
NB = 512  # rows per buffer
NS = NB // 128  # subtiles per buffer
LN_EPS = 1e-5


def _ensure_axon_hooks():
    """Register the NTFF profile hook if the image's antenv lacks axon_hooks.

    Without this, trace=True degrades to no profiling (runs still work)."""
    try:
        import antenv  # noqa: F401
        from antenv import axon_hooks  # noqa: F401
        return
    except ImportError:
        pass
    try:
        import antenv
        mod = types.ModuleType("antenv.axon_hooks")
        mod._hook = None
        mod.set_axon_ntff_profile_hook = lambda h: setattr(mod, "_hook", h)
        mod.get_axon_ntff_profile_hook = lambda: mod._hook
        sys.modules["antenv.axon_hooks"] = mod
        antenv.axon_hooks = mod
        from trn_agent_boot.trn_boot import _ntff_profile_via_ctypes
        mod.set_axon_ntff_profile_hook(
            _ntff_profile_via_ctypes("/opt/axon/libaxon_pjrt.so")
        )
    except Exception:
        pass


def make_consts(W_int, W_stim, trans, ln_w, ln_b, W_out, b_out):
    """Host-side constant tensors (all fp32)."""
    W_int = np.asarray(W_int, np.float32)
    W_stim = np.asarray(W_stim, np.float32)
    trans = np.asarray(trans, np.float32)
    ln_w = np.asarray(ln_w, np.float32)
    ln_b = np.asarray(ln_b, np.float32)
    W_out = np.asarray(W_out, np.float32)
    b_out = np.asarray(b_out, np.float32)

    # Projection weights: Y[:, 0:9] = this = feat[:,1,:] @ W_stim.T
    #                     Y[:, 32:41] = last = feat[:,0,:] @ W_int.T
    W_cat = np.zeros((D, 41), np.float32)
    W_cat[768:1536, 0:9] = W_stim.T
    W_cat[0:768, 32:41] = W_int.T
    # Scramble for chunked contraction: chunk c partition i holds d = c*128+i
    import ml_dtypes
    ws = np.zeros((128, 12, 41), np.float32)
    for c in range(12):
        ws[:, c, :] = W_cat[c * 128:(c + 1) * 128, :]
    ws = ws.astype(ml_dtypes.bfloat16)

    # trans matrix for G[a*9+k, b] = sum_j trans[a,j,k] * last[j, b]
    # rows live at partitions 32:41 to match last's position in Y.
    tm = np.zeros((41, 81), np.float32)
    for a in range(9):
        for j in range(9):
            for k in range(9):
                tm[32 + j, a * 9 + k] = trans[a, j, k]

    # thisbc[a*9+k, b] = this[a, b]
    e9 = np.zeros((9, 81), np.float32)
    for a in range(9):
        e9[a, a * 9:(a + 1) * 9] = 1.0

    # bil_centered[k', b] = sum_a M[a*9+k', b] - (1/9) sum_rows M[row, b]
    rp = np.full((81, 9), -1.0 / 9.0, np.float32)
    for a in range(9):
        for k in range(9):
            rp[a * 9 + k, k] += 1.0

    o99 = np.full((9, 1), 1.0 / 9.0, np.float32)   # mean-of-squares reducer
    o19 = np.ones((1, 9), np.float32)              # rstd partition-broadcast

    # Final linear with ln_w/ln_b folded in:
    # out = W_out[:, :9] @ this + (W_out[:, 9:] * ln_w) @ (bil_c * rstd) + b'
    l1 = np.ascontiguousarray(W_out[:, 0:9].T)
    l2 = np.ascontiguousarray((W_out[:, 9:18] * ln_w[None, :]).T)
    bout = (b_out + W_out[:, 9:18] @ ln_b).reshape(9, 1).astype(np.float32)

    i128 = np.eye(128, dtype=ml_dtypes.bfloat16)
    i9 = np.eye(9, dtype=np.float32)

    return {
        "ws": ws, "tm": tm, "e9": e9, "rp": rp, "o99": o99, "o19": o19,
        "l1": l1, "l2": l2, "bout": bout, "i128": i128, "i9": i9,
        "eps": np.full((1, 1), LN_EPS, np.float32),
    }


def build_program(b_core=B_CORE, num_devices=N_CORES):
    import concourse.bass as bass  # noqa: F401
    import concourse.tile as tile
    from concourse import bacc, mybir

    f32 = mybir.dt.float32
    bf16 = mybir.dt.bfloat16
    nc = bacc.Bacc("TRN2", target_bir_lowering=False, debug=False,
                   num_devices=num_devices)

    feat_d = nc.dram_tensor("feat", [b_core, D], f32, kind="ExternalInput")
    out_d = nc.dram_tensor("out", [b_core, 9], f32, kind="ExternalOutput")
    cshapes = {
        "ws": [128, 12, 41], "tm": [41, 81], "e9": [9, 81], "rp": [81, 9],
        "o99": [9, 1], "o19": [1, 9], "l1": [9, 9], "l2": [9, 9],
        "bout": [9, 1], "i128": [128, 128], "i9": [9, 9], "eps": [1, 1],
    }
    cddt = {"ws": bf16, "i128": bf16}
    cd = {k: nc.dram_tensor(k, v, cddt.get(k, f32), kind="ExternalInput")
          for k, v in cshapes.items()}

    nbuf = b_core // NB
    with tile.TileContext(nc) as tc:
        with tc.tile_pool(name="consts", bufs=1) as cp, \
             tc.tile_pool(name="nat", bufs=2) as natp, \
             tc.tile_pool(name="natb", bufs=2) as natbp, \
             tc.tile_pool(name="ft", bufs=2) as ftp, \
             tc.tile_pool(name="ysb", bufs=2) as ysbp, \
             tc.tile_pool(name="episb", bufs=6) as esbp, \
             tc.tile_pool(name="outsb", bufs=2) as outp, \
             tc.tile_pool(name="trps", bufs=2, space="PSUM") as trp, \
             tc.tile_pool(name="yps", bufs=2, space="PSUM") as yp, \
             tc.tile_pool(name="epips", bufs=3, space="PSUM") as epp:

            cdt = {"ws": bf16, "i128": bf16}
            cs = {k: cp.tile(v, cdt.get(k, f32), tag=k, name=k)
                  for k, v in cshapes.items()}
            for k in cshapes:
                nc.sync.dma_start(cs[k][:], cd[k].ap())

            for ib in range(nbuf):
                rows = feat_d.ap()[ib * NB:(ib + 1) * NB, :]
                nat = natp.tile([128, NS, D], f32, tag="nat")
                nc.sync.dma_start(nat[:], rows.rearrange("(s p) d -> p s d", p=128))
                # fp32 -> bf16 cast on the otherwise-idle GpSimd engine
                natb = natbp.tile([128, NS, D], bf16, tag="natb")
                for s in range(NS):
                    nc.gpsimd.tensor_copy(natb[:, s, :], nat[:, s, :])

                ft = ftp.tile([128, 12, NB], bf16, tag="ft")
                y_ps = yp.tile([41, NB], f32, tag="y")
                # 2-chunk groups: 8 transposes -> 1 wide copy -> 2 proj MMs,
                # so real (HAM-warming) matmuls interleave with transposes
                for t in range(6):
                    tr = trp.tile([128, 1024], bf16, tag="tr")
                    for k in range(8):
                        idx = t * 8 + k
                        c, s = idx // 4, idx % 4
                        nc.tensor.matmul(
                            tr[:, (c % 2) * 512 + s * 128:(c % 2) * 512 + (s + 1) * 128],
                            natb[:, s, c * 128:(c + 1) * 128],
                            cs["i128"][:],
                            is_transpose=True,
                            start=(k == 0), stop=(k == 7),
                        )
                    if t % 2 == 0:
                        nc.vector.tensor_copy(ft[:, 2 * t:2 * t + 2, :], tr[:])
                    else:
                        nc.scalar.copy(ft[:, 2 * t:2 * t + 2, :], tr[:])
                    for c in (2 * t, 2 * t + 1):
                        nc.tensor.matmul(
                            y_ps[:],
                            cs["ws"][:, c, :],
                            ft[:, c, :],
                            start=(c == 0),
                            stop=(c == 11),
                        )

                # ---- epilogue: bilinear + LN + final linear, [*, NB] ----
                y_sb = ysbp.tile([41, NB], f32, tag="y_sb")
                nc.scalar.copy(y_sb[:], y_ps[:])
                g_ps = epp.tile([81, NB], f32, tag="ep")
                nc.tensor.matmul(g_ps[:], cs["tm"][32:41, :], y_sb[32:41, :],
                                 tile_position=(32, 0))
                tb_ps = epp.tile([81, NB], f32, tag="ep")
                nc.tensor.matmul(tb_ps[:], cs["e9"][:], y_sb[0:9, :])
                tb_sb = esbp.tile([81, NB], f32, tag="ep_sb")
                nc.scalar.copy(tb_sb[:], tb_ps[:])
                m_sb = esbp.tile([81, NB], f32, tag="ep_sb")
                nc.vector.tensor_mul(m_sb[:], g_ps[:], tb_sb[:])
                bil_ps = epp.tile([9, NB], f32, tag="ep")
                nc.tensor.matmul(bil_ps[:], cs["rp"][:], m_sb[:])
                bil_sb = esbp.tile([9, NB], f32, tag="ep_sb")
                nc.scalar.copy(bil_sb[:], bil_ps[:])
                sq_sb = esbp.tile([9, NB], f32, tag="ep_sb")
                nc.scalar.square(sq_sb[:], bil_sb[:])
                var_ps = epp.tile([1, NB], f32, tag="ep")
                nc.tensor.matmul(var_ps[:], cs["o99"][:], sq_sb[:])
                lnv_sb = esbp.tile([1, NB], f32, tag="ep_sb")
                nc.scalar.activation(lnv_sb[:], var_ps[:],
                                     mybir.ActivationFunctionType.Ln,
                                     bias=cs["eps"][:, 0:1])
                rstd_sb = esbp.tile([1, NB], f32, tag="ep_sb")
                nc.scalar.activation(rstd_sb[:], lnv_sb[:],
                                     mybir.ActivationFunctionType.Exp,
                                     scale=-0.5)
                rb_ps = epp.tile([9, NB], f32, tag="ep")
                nc.tensor.matmul(rb_ps[:], cs["o19"][:], rstd_sb[:])
                ln_sb = esbp.tile([9, NB], f32, tag="ep_sb")
                nc.vector.tensor_mul(ln_sb[:], rb_ps[:], bil_sb[:])
                o_ps = epp.tile([9, NB], f32, tag="ep")
                nc.tensor.matmul(o_ps[:], cs["l2"][:], ln_sb[:],
                                 start=True, stop=False)
                nc.tensor.matmul(o_ps[:], cs["l1"][:], y_sb[0:9, :],
                                 start=False, stop=True)
                osb = esbp.tile([9, NB], f32, tag="ep_sb")
                nc.vector.tensor_scalar_add(osb[:], o_ps[:], cs["bout"][:, 0:1])

                ot_ps = epp.tile([128, NS * 9], f32, tag="ep")
                for s in range(NS):
                    nc.tensor.matmul(
                        ot_ps[:, s * 9:(s + 1) * 9],
                        osb[:, s * 128:(s + 1) * 128],
                        cs["i9"][:],
                        is_transpose=True,
                        start=(s == 0), stop=(s == NS - 1),
                    )
                out_sb = outp.tile([128, NS, 9], f32, tag="out_sb")
                nc.vector.tensor_copy(out_sb[:], ot_ps[:])
                nc.sync.dma_start(
                    out_d.ap()[ib * NB:(ib + 1) * NB, :]
                    .rearrange("(s p) k -> p s k", p=128),
                    out_sb[:],
                )
    nc.compile()
    return nc


_PROGRAM = None


def _get_program():
    global _PROGRAM
    if _PROGRAM is None:
        _PROGRAM = build_program()
    return _PROGRAM


def kernel(feat, W_int, W_stim, trans, ln_w, ln_b, W_out, b_out,
           trace=False, trace_kwargs=None):
    _ensure_axon_hooks()
    from concourse.bass_utils import run_bass_kernel_spmd

    feat = np.asarray(feat, np.float32)
    feat2 = feat.reshape(B_FULL, D)
    consts = make_consts(W_int, W_stim, trans, ln_w, ln_b, W_out, b_out)
    nc = _get_program()
    in_maps = []
    for c in range(N_CORES):
        m = {"feat": np.ascontiguousarray(feat2[c * B_CORE:(c + 1) * B_CORE])}
        m.update(consts)
        in_maps.append(m)
    res = run_bass_kernel_spmd(nc, in_maps, list(range(N_CORES)), trace=trace)
    out = np.concatenate([res.results[c]["out"] for c in range(N_CORES)], axis=0)
    kernel.last_results = res
    return np.ascontiguousarray(out, dtype=np.float32)
